# revision 48
# baseline (speedup 1.0000x reference)
"""Trainium2 Bass kernel for the GNN message-passing module.

Reference computation (per batch b):
    msg_n = node @ Wn + bn                      (N, MID)
    msg_h = hidden @ Wh + bh                    (N, MID)
    msg_e = edge @ We + be                      (N, N, MID)
    msg_g = graph @ Wg + bg                     (MID,)
    msgs[i,j,:] = msg_n[j] + msg_h[i] + msg_e[i,j] + msg_g
    out_msgs[j,:] = max_i(msgs[i,j,:] * adj[i,j])
    ret = node @ Wo1 + bo1 + hidden @ Wo2 + bo2 + out_msgs @ Wo3 + bo3

Kernel strategy (data-parallel, one batch per core across 8 cores):
  - Orientation: channels on SBUF partitions, j (receiver) on the free dim.
  - edge is pre-packed on the host to bf16 in (group, e, a, j) layout with
    GI=32 sender rows per group, so each group loads as ONE fully
    contiguous 2MB DMA (16KB per partition) -> near-peak HBM bandwidth,
    and half the bytes of fp32.
  - The multiplicative {0,1} adjacency mask is converted to an additive mask
    adjm = (adj-1)*1e30 in {0, -1e30} (bf16), folded into the PE
    accumulation as a rank-1 matmul (ones_c (x) adjm_row_i).  A per-j
    correction vector cvec restores the exact max semantics.
  - msg_n is constant in i, so it is pulled out of the max and added once.
  - h_i = msg_h[i] + msg_g + (bn+bh+be+bg) enters via the same rank-3
    matmul ([h_even; h_odd; ones] x [sel0; sel1; adjm-pair]), two sender
    rows per 512-wide PSUM half.
  - DVE does one wide (128,1024) running max per 4 sender rows.
"""

from contextlib import ExitStack

import numpy as np

B, N, D, E, G, MID, OUT = 8, 256, 128, 128, 128, 128, 128
NCORES = 8
BIG = 1.0e30
GI = 32            # sender rows (i values) per edge group / DMA
NG = N // GI       # 8 groups
NPAIR = GI // 2    # 16 pairs per group
NTILE = GI // 4    # 8 PSUM tiles per group (4 rows each)
NT = N // 128      # number of 128-row tiles along N

_WNAMES = ["Wn", "Wh", "We", "Wg", "Wo1", "Wo2", "Wo3"]
_BNAMES = ["bn", "bh", "be", "bg", "bo1", "bo2", "bo3"]

_CACHE = {}


def _ensure_path():
    try:
        import concourse.bass  # noqa: F401
    except ImportError:
        import sys

        for p in ("/opt/trn_rl_repo", "/root/.axon_site/_ro/trn_rl_repo"):
            if p not in sys.path:
                sys.path.insert(0, p)
        import concourse.bass  # noqa: F401
    _patch_ldw_opt()


def _patch_ldw_opt():
    """No-op: walrus ldw-opt rejects the standalone InstLdweights that
    legalization inserts for bf16 matmuls; keep the default (off)."""
    return


def _dedup_ldweights(m):
    """Remove back-to-back duplicate PE LDWEIGHTS.

    Legalization emits one InstLdweights per (non-f32) matmul; consecutive
    matmuls on the same stationary operand only need the first.  Runs
    pre-compile, when the redundant LDWs carry no sync info (bacc's
    move_matmul_waits_to_ldweights hoists matmul waits onto the surviving
    LDW afterwards; within a run all matmuls wait on the same staging
    semaphores, so the coarsening is harmless)."""
    n = 0
    for fn in m.functions:
        for blk in fn.blocks:
            last = None
            doomed = []
            for inst in list(blk.instructions):
                if str(getattr(inst, "engine", "")) != "EngineType.PE":
                    continue
                nm = type(inst).__name__
                if nm == "InstLdweights":
                    si = inst.sync_info
                    clean = si is None or (not si.on_wait and not si.on_update)
                    sig = (
                        repr(inst.ins[0]),
                        str(inst.perf_mode),
                        str(inst.tile_position),
                        str(inst.is_transpose),
                    )
                    if sig == last and clean:
                        doomed.append(inst)
                    else:
                        last = sig
                elif nm == "InstMatmult":
                    continue
                else:
                    last = None
            for inst in doomed:
                blk.instructions.remove(inst)
            n += len(doomed)
    return n


def _kernel_body(ctx, tc, aps, rep=0, edge_groups=None, ablate=()):
    import concourse.bass as bass  # noqa: F401
    from concourse import masks, mybir

    nc = tc.nc
    f32 = mybir.dt.float32
    f32r = mybir.dt.float32r
    bf16 = mybir.dt.bfloat16
    Alu = mybir.AluOpType

    edge = aps["edge"]
    node = aps["node"]
    hidden = aps["hidden"]
    graph = aps["graph"]
    adj = aps["adj"]
    out = aps["out"]

    const = ctx.enter_context(tc.tile_pool(name="const", bufs=1))
    opool = ctx.enter_context(tc.tile_pool(name="op", bufs=4, space="PSUM"))
    ps_pool = opool
    scratch = ctx.enter_context(tc.tile_pool(name="scratch", bufs=1))
    epool = ctx.enter_context(tc.tile_pool(name="edgein", bufs=3))

    # ---- constants -------------------------------------------------------
    ident = const.tile([128, 128], f32)
    masks.make_identity(nc, ident[:])

    ones_f = scratch.tile([1, 256], f32)
    nc.vector.memset(ones_f[:], 1.0)
    ones_row = const.tile([1, 256], f32r)
    nc.vector.tensor_copy(ones_row[:], ones_f[:])
    ones_1c = const.tile([1, 128], f32r)
    nc.vector.tensor_copy(ones_1c[:], ones_f[:, 0:128])
    ones_11 = const.tile([1, 1], f32r)
    nc.vector.tensor_copy(ones_11[:], ones_f[:, 0:1])
    ones_colf = scratch.tile([128, 1], f32)
    nc.vector.memset(ones_colf[:], 1.0)
    ones_col = const.tile([128, 1], bf16)
    nc.vector.tensor_copy(ones_col[:], ones_colf[:])

    W_sb = {}
    We_bf = const.tile([128, 128], bf16, name=f"r{rep}_We_bf", tag="We_bf")
    for w in _WNAMES:
        Wf = scratch.tile([128, 128], f32, name=f"r{rep}_Wf_{w}", tag=f"Wf_{w}")
        nc.sync.dma_start(Wf[:], aps[w])
        W_sb[w] = const.tile([128, 128], f32r, name=f"r{rep}_W_{w}", tag=f"W_{w}")
        nc.vector.tensor_copy(W_sb[w][:], Wf[:])
        if w == "We":
            nc.vector.tensor_copy(We_bf[:], Wf[:])
    B_sb = {}
    for b in _BNAMES:
        Bf = scratch.tile([1, 128], f32, name=f"r{rep}_Bf_{b}", tag=f"Bf_{b}")
        nc.sync.dma_start(Bf[:], aps[b].rearrange("(o k) -> o k", o=1))
        B_sb[b] = const.tile([1, 128], f32r, name=f"r{rep}_B_{b}", tag=f"B_{b}")
        nc.vector.tensor_copy(B_sb[b][:], Bf[:])

    graph_colf = scratch.tile([128, 1], f32)
    nc.sync.dma_start(graph_colf[:], graph.rearrange("(p o) -> p o", o=1))
    graph_col = const.tile([128, 1], f32r)
    nc.vector.tensor_copy(graph_col[:], graph_colf[:])

    node_nat = scratch.tile([128, NT * 128], f32)
    nc.sync.dma_start(
        node_nat[:].rearrange("p (t d) -> p t d", t=NT),
        node.rearrange("(t p) d -> p t d", p=128),
    )
    hid_nat = scratch.tile([128, NT * 128], f32)
    nc.sync.dma_start(
        hid_nat[:].rearrange("p (t d) -> p t d", t=NT),
        hidden.rearrange("(t p) d -> p t d", p=128),
    )
    adj_nat = scratch.tile([128, NT * 256], mybir.dt.int32)
    nc.sync.dma_start(
        adj_nat[:].rearrange("p (t j) -> p t j", t=NT),
        adj.rearrange("(t p) j -> p t j", p=128),
    )

    # ---- per-batch precompute -------------------------------------------
    # nodeT / hidT: (d, n) layouts via PE transpose
    nodeT = const.tile([128, 256], f32r)
    hidT = const.tile([128, 256], f32r)
    for nat, T in ((node_nat, nodeT), (hid_nat, hidT)):
        ps = ps_pool.tile([128, 256], f32, tag="op")
        for t in range(NT):
            nc.tensor.transpose(
                ps[:, t * 128 : (t + 1) * 128],
                nat[:, t * 128 : (t + 1) * 128],
                ident[:],
            )
        nc.scalar.copy(T[:], ps[:])

    # r0 = graph @ Wg + (bn + bh + be + bg), a (1, MID) row
    ps_r0 = ps_pool.tile([128, 256], f32, tag="op")
    nc.tensor.matmul(
        ps_r0[0:1, 0:128],
        graph_col[:],
        W_sb["Wg"][:],
        start=True,
        stop=False,
    )
    for k, bname in enumerate(["bn", "bh", "be", "bg"]):
        nc.tensor.matmul(
            ps_r0[0:1, 0:128],
            ones_11[:],
            B_sb[bname][:],
            start=False,
            stop=(k == 3),
        )
    r0 = const.tile([1, 128], f32r)
    nc.scalar.copy(r0[:], ps_r0[0:1, 0:128])

    # H_nat[i, c] = hidden @ Wh + r0  (h_i rows, natural orientation)
    ps_h = ps_pool.tile([128, 256], f32, tag="op")
    for t in range(NT):
        nc.tensor.matmul(
            ps_h[:, t * 128 : (t + 1) * 128],
            hidT[:, t * 128 : (t + 1) * 128],
            W_sb["Wh"][:],
            start=True,
            stop=False,
        )
        nc.tensor.matmul(
            ps_h[:, t * 128 : (t + 1) * 128],
            ones_1c[:],
            r0[:],
            start=False,
            stop=True,
        )
    H_bf = scratch.tile([128, 256], bf16)
    nc.scalar.copy(H_bf[:], ps_h[:])

    # h_dram layout [G, p16, u2, c]: group G = t*4 + g4, pair p, member u
    # (i = 32G + 2p + u).  Store partition pp = g4*32 + 2p + u = (g p u),
    # which is an adjacent nested group of this layout.
    h_dram = aps["h_scratch"]
    a_dram = aps["a_scratch"]
    nc.sync.dma_start(
        h_dram.rearrange("(t g p u c) -> (g p u) t c", t=NT, g=4, p=16, u=2),
        H_bf[:].rearrange("p (t c) -> p t c", t=NT),
    )

    # msg_nT[c, j] = (node @ Wn).T  (no bias: biases live in r0)
    ps_mn = ps_pool.tile([128, 256], f32, tag="op")
    nc.tensor.matmul(
        ps_mn[:], W_sb["Wn"][:], nodeT[:],
        start=True, stop=True,
    )
    msg_nT = const.tile([128, 256], f32)
    nc.scalar.copy(msg_nT[:], ps_mn[:])

    # adjm = (adj - 1) * BIG  in {0, -BIG}, bf16, natural layout; write back
    # row-major so per-group slices are contiguous.
    adj_f = scratch.tile([128, NT * 256], f32)
    nc.vector.tensor_copy(adj_f[:], adj_nat[:])
    adjm = scratch.tile([128, NT * 256], bf16)
    nc.vector.tensor_scalar(adjm[:], adj_f[:], -1.0, BIG, Alu.add, Alu.mult)
    nc.sync.dma_start(
        a_dram.rearrange("(t p) j -> p t j", p=128),
        adjm[:].rearrange("p (t j) -> p t j", t=NT),
    )

    # cvec[j] = -BIG if column fully kept (sum_i adjm == 0), else 0 (the
    # "0 candidate" of the reference max)
    ps_s = ps_pool.tile([128, 256], f32, tag="op")
    for t in range(NT):
        nc.tensor.matmul(
            ps_s[0:1, :],
            ones_col[:],
            adjm[:, t * 256 : (t + 1) * 256],
            start=(t == 0),
            stop=(t == NT - 1),
        )
    cvec = const.tile([1, 256], f32r)
    nc.vector.tensor_scalar(cvec[:], ps_s[0:1, :], -1.0e29, -BIG, Alu.is_ge, Alu.mult)

    # Persistent double-buffered staging tiles for the fused K=3 matmul.
    # Per pair p (rows i=32G+2p, 32G+2p+1):
    #   Hab block [3,128]: [h_even; h_odd; ones]
    #   AR3 block [3,512]: [sel0; sel1; adjm_even | adjm_odd]
    habA = const.tile([3, NPAIR * 128], bf16)
    habB = const.tile([3, NPAIR * 128], bf16)
    arA = const.tile([3, NPAIR * 512], bf16)
    arB = const.tile([3, NPAIR * 512], bf16)
    # selpat = (ones256 zeros256) x (NPAIR+1); sel0 = selpat[0:NPAIR*512],
    # sel1 = selpat[256 : 256+NPAIR*512].  Built at partition 0, DMA'd into
    # rows 0/1 (compute engines cannot address base partitions > 0).
    selpat = scratch.tile([1, (NPAIR + 1) * 512], bf16)
    nc.vector.memset(selpat[:], 0.0)
    nc.vector.memset(
        selpat[:].rearrange("o (b u j) -> o b u j", b=NPAIR + 1, u=2)[:, :, 0:1, :],
        1.0,
    )
    sel0f = selpat[:, 0 : NPAIR * 512]
    sel1f = selpat[:, 256 : 256 + NPAIR * 512]
    onesw_f = scratch.tile([1, NPAIR * 128], bf16)
    nc.vector.memset(onesw_f[:], 1.0)
    for dst in (arA, arB):
        nc.gpsimd.dma_start(dst[0:1, :], sel0f)
        nc.gpsimd.dma_start(dst[1:2, :], sel1f)
    for dst in (habA, habB):
        nc.gpsimd.dma_start(dst[2:3, :], onesw_f[:])

    # running max accumulators (channels x (chunk, j)).  The PSUM drain is
    # split across engines: tile 0 of each half goes DVE-direct (fp32 max
    # from PSUM); tiles 1-3 are evacuated by ACT (PSUM -> SBUF bf16 copy)
    # and maxed by GPSIMD from SBUF.  Chunk alignment (a mod 4) is the same
    # for every tile, so any distribution over accumulators is sound.
    accs = []
    for q in range(2):
        a_ = const.tile([128, 1024], f32, name=f"r{rep}_acc{q}", tag=f"acc{q}")
        nc.vector.memset(a_[:], -3.0e38)
        accs.append(a_)
    gaccs = []
    for q in range(3):
        a_ = const.tile([128, 1024], bf16, name=f"r{rep}_gacc{q}", tag=f"gacc{q}")
        nc.vector.memset(a_[:], -3.0e38)
        gaccs.append(a_)
    stpool = ctx.enter_context(tc.tile_pool(name="st", bufs=4))

    # ---- main loop over sender-row groups -------------------------------
    h_view = h_dram.rearrange("(G p u c) -> G p u c", G=NG, p=16, u=2)
    a_view = a_dram.rearrange("(G a) j -> G (a j)", G=NG)
    edge_v = edge.rearrange("g p a j -> p g a j")

    def stage_a(g):
        """Load edge group g: one fully-contiguous 2MB bf16 DMA."""
        gsrc = g if edge_groups is None else (g % edge_groups)
        et = epool.tile([128, GI * 256], bf16, tag="et", name=f"r{rep}_et{g}")
        nc.sync.dma_start(
            et[:].rearrange("p (a j) -> p a j", a=GI),
            edge_v[:, gsrc],
        )
        return et

    def stage_h(g):
        """Stage h-pair rows + adjm rows for group g (ACT HWDGE ring)."""
        AR3, Hab = (arA, habA) if g % 2 == 0 else (arB, habB)
        nc.scalar.dma_start(
            Hab[0:2, :].rearrange("u (o p c) -> u o p c", o=1, p=16),
            h_view[g : g + 1].transpose([2, 0, 1, 3]),
        )
        nc.scalar.dma_start(AR3[2:3, :], a_view[g : g + 1])
        return (AR3, Hab)

    no_we = "we" in ablate
    no_mask = "mask" in ablate
    no_dve = "dve" in ablate

    def stage_b(g, et, chunk):
        """msg_e matmuls + fused (h, adjm) rank-3 matmuls + wide running max.

        Processed in half-group batches of 4 PSUM tiles (8 banks) so all 8
        We matmuls run back-to-back: walrus ldw-opt dedupes the identical
        We weight loads into one."""
        AR3, Hab = chunk
        if no_we and no_mask:
            return
        for half in range(2):
            t0 = half * (NTILE // 2)
            ops = []
            for t in range(t0, t0 + NTILE // 2):
                ops.append(
                    opool.tile([128, 1024], f32, tag="op", name=f"r{rep}_op{g}_{t}")
                )
            if not no_we:
                for t in range(t0, t0 + NTILE // 2):
                    for hh in range(2):
                        p = 2 * t + hh
                        nc.tensor.matmul(
                            ops[t - t0][:, hh * 512 : (hh + 1) * 512],
                            We_bf[:],
                            et[:, p * 512 : (p + 1) * 512],
                            start=True, stop=no_mask,
                        )
            if not no_mask:
                for t in range(t0, t0 + NTILE // 2):
                    for hh in range(2):
                        p = 2 * t + hh
                        nc.tensor.matmul(
                            ops[t - t0][:, hh * 512 : (hh + 1) * 512],
                            Hab[0:3, p * 128 : (p + 1) * 128],
                            AR3[0:3, p * 512 : (p + 1) * 512],
                            start=no_we, stop=True,
                        )
            if not no_dve:
                for t in range(t0, t0 + NTILE // 2):
                    op = ops[t - t0]
                    k = t % 4
                    if k == 0:
                        a_ = accs[t // 4]
                        nc.vector.tensor_tensor(a_[:], op[:], a_[:], Alu.max)
                    else:
                        st = stpool.tile(
                            [128, 1024], bf16, tag="st", name=f"r{rep}_st{g}_{t}"
                        )
                        nc.scalar.copy(st[:], op[:])
                        ga = gaccs[k - 1]
                        nc.vector.tensor_tensor(ga[:], st[:], ga[:], Alu.max)

    if "loop" not in ablate:
        no_sh = "stageh" in ablate
        prev = None
        for g in range(NG):
            ck = (arA, habA) if no_sh else stage_h(g)
            et = stage_a(g)
            if prev is not None:
                stage_b(prev[0], prev[1], prev[2])
            prev = (g, et, ck)
        stage_b(prev[0], prev[1], prev[2])

    # ---- finalize --------------------------------------------------------
    a01 = const.tile([128, 1024], f32)
    nc.vector.tensor_tensor(a01[:], accs[0][:], accs[1][:], Alu.max)
    g01 = const.tile([128, 1024], bf16)
    nc.vector.tensor_tensor(g01[:], gaccs[0][:], gaccs[1][:], Alu.max)
    g012 = const.tile([128, 1024], bf16)
    nc.vector.tensor_tensor(g012[:], g01[:], gaccs[2][:], Alu.max)
    gf = const.tile([128, 1024], f32)
    nc.vector.tensor_copy(gf[:], g012[:])
    aw = const.tile([128, 1024], f32)
    nc.vector.tensor_tensor(aw[:], a01[:], gf[:], Alu.max)
    ah = const.tile([128, 512], f32)
    nc.vector.tensor_tensor(ah[:], aw[:, 0:512], aw[:, 512:1024], Alu.max)
    acc = const.tile([128, 256], f32)
    nc.vector.tensor_tensor(acc[:], ah[:, 0:256], ah[:, 256:512], Alu.max)

    ps_cv = ps_pool.tile([128, 256], f32, tag="op")
    nc.tensor.matmul(
        ps_cv[:], ones_1c[:], cvec[:],
        start=True, stop=True,
    )
    msgsT = const.tile([128, 256], f32)
    nc.vector.tensor_tensor(msgsT[:], acc[:], msg_nT[:], Alu.add)
    resT = const.tile([128, 256], f32r)
    nc.vector.tensor_tensor(resT[:], msgsT[:], ps_cv[:], Alu.max)

    # ret_T (o, n)
    ps_ret = ps_pool.tile([128, 256], f32, tag="op")
    nc.tensor.matmul(
        ps_ret[:], W_sb["Wo1"][:], nodeT[:],
        start=True, stop=False,
    )
    nc.tensor.matmul(
        ps_ret[:], W_sb["Wo2"][:], hidT[:],
        start=False, stop=False,
    )
    nc.tensor.matmul(
        ps_ret[:], W_sb["Wo3"][:], resT[:],
        start=False, stop=False,
    )
    for k, bname in enumerate(["bo1", "bo2", "bo3"]):
        nc.tensor.matmul(
            ps_ret[:],
            B_sb[bname][:],
            ones_row[:],
            start=False,
            stop=(k == 2),
        )
    retT = const.tile([128, 256], f32)
    nc.scalar.copy(retT[:], ps_ret[:])

    ps_out = ps_pool.tile([128, 256], f32, tag="op")
    for t in range(NT):
        nc.tensor.transpose(
            ps_out[:, t * 128 : (t + 1) * 128],
            retT[:, t * 128 : (t + 1) * 128],
            ident[:],
        )
    out_sb = const.tile([128, 256], f32)
    nc.scalar.copy(out_sb[:], ps_out[:])
    nc.sync.dma_start(
        out.rearrange("(t p) o -> p t o", p=128),
        out_sb[:].rearrange("p (t o) -> p t o", t=NT),
    )


def build_nc(repeat=1, edge_groups=None, loop_n=1, ablate=()):
    """Build the (single-core SPMD) Bass program; returns nc."""
    _ensure_path()
    import concourse.tile as tile
    from concourse import bacc, mybir

    f32 = mybir.dt.float32
    i32 = mybir.dt.int32
    bf16 = mybir.dt.bfloat16

    nc = bacc.Bacc(
        "TRN2", target_bir_lowering=False, debug=False, num_devices=NCORES
    )
    n_groups = NG if edge_groups is None else edge_groups
    aps = {
        "edge": nc.dram_tensor(
            "edge", [n_groups, E, GI, N], bf16, kind="ExternalInput"
        ).ap(),
        "node": nc.dram_tensor("node", [N, D], f32, kind="ExternalInput").ap(),
        "hidden": nc.dram_tensor("hidden", [N, D], f32, kind="ExternalInput").ap(),
        "graph": nc.dram_tensor("graph", [G], f32, kind="ExternalInput").ap(),
        "adj": nc.dram_tensor("adj", [N, N], i32, kind="ExternalInput").ap(),
        "out": nc.dram_tensor("out", [N, OUT], f32, kind="ExternalOutput").ap(),
    }
    for w in _WNAMES:
        aps[w] = nc.dram_tensor(w, [128, 128], f32, kind="ExternalInput").ap()
    for b in _BNAMES:
        aps[b] = nc.dram_tensor(b, [128], f32, kind="ExternalInput").ap()
    aps["h_scratch"] = nc.dram_tensor("h_scratch", [N * MID], bf16).ap()
    aps["a_scratch"] = nc.dram_tensor("a_scratch", [N, N], bf16).ap()

    with tile.TileContext(nc) as tc:
        if loop_n > 1:
            with tc.For_i(0, loop_n, 1):
                with ExitStack() as ctx:
                    _kernel_body(
                        ctx, tc, aps, rep=0, edge_groups=edge_groups, ablate=ablate
                    )
        else:
            for rep in range(repeat):
                with ExitStack() as ctx:
                    _kernel_body(
                        ctx, tc, aps, rep=rep, edge_groups=edge_groups, ablate=ablate
                    )
    _dedup_ldweights(nc.m)
    nc.compile()
    return nc


def _get_nc():
    if "nc" not in _CACHE:
        _CACHE["nc"] = build_nc()
    return _CACHE["nc"]


def _pack_edge(e):
    """(N, N, E) f32 -> (NG, E, GI, N) bf16, fully contiguous per group."""
    import ml_dtypes

    x = np.asarray(e, np.float32).astype(ml_dtypes.bfloat16)  # (i, j, e)
    x = x.transpose(0, 2, 1)                                  # (i, e, j)
    x = x.reshape(NG, GI, E, N).transpose(0, 2, 1, 3)         # (g, e, a, j)
    return np.ascontiguousarray(x)


def make_in_maps(**inputs):
    """Shard full inputs into per-core input maps (batch-parallel)."""
    in_maps = []
    for c in range(NCORES):
        m = {
            "edge": _pack_edge(inputs["edge_fts"][c]),
            "node": np.ascontiguousarray(inputs["node_fts"][c], np.float32),
            "hidden": np.ascontiguousarray(inputs["hidden"][c], np.float32),
            "graph": np.ascontiguousarray(inputs["graph_fts"][c], np.float32),
            "adj": np.ascontiguousarray(inputs["adj_mat"][c], np.int32),
        }
        for w in _WNAMES:
            m[w] = np.ascontiguousarray(inputs[w], np.float32)
        for b in _BNAMES:
            m[b] = np.ascontiguousarray(inputs[b], np.float32)
        in_maps.append(m)
    return in_maps


def kernel(**inputs) -> np.ndarray:
    """Full-input entry point: shards over 8 cores, returns (B, N, OUT)."""
    _ensure_path()
    from concourse import bass_utils

    nc = _get_nc()
    in_maps = make_in_maps(**inputs)
    res = bass_utils.run_bass_kernel_spmd(nc, in_maps, core_ids=list(range(NCORES)))
    outs = [res.results[c]["out"] for c in range(NCORES)]
    return np.stack(outs, axis=0).astype(np.float32)


def kernel_traced(tmpdir=None, **inputs):
    """Like kernel(), but requests an NTFF profile; returns (out, results)."""
    _ensure_path()
    from concourse import bass_utils

    nc = _get_nc()
    in_maps = make_in_maps(**inputs)
    res = bass_utils.run_bass_kernel_spmd(
        nc, in_maps, core_ids=list(range(NCORES)), trace=True, tmpdir=tmpdir
    )
    outs = [res.results[c]["out"] for c in range(NCORES)]
    return np.stack(outs, axis=0).astype(np.float32), res


if __name__ == "__main__":
    rng = np.random.default_rng(0)
    inputs = {
        "node_fts": rng.normal(size=(B, N, D)).astype(np.float32),
        "edge_fts": rng.normal(size=(B, N, N, E)).astype(np.float32),
        "graph_fts": rng.normal(size=(B, G)).astype(np.float32),
        "adj_mat": rng.integers(0, 2, size=(B, N, N)).astype(np.int32),
        "hidden": rng.normal(size=(B, N, D)).astype(np.float32),
    }
    s = 0.02
    for w in _WNAMES:
        inputs[w] = (s * rng.normal(size=(128, 128))).astype(np.float32)
    for b in _BNAMES:
        inputs[b] = np.zeros(128, np.float32)
    out = kernel(**inputs)
    print(out.shape, out.dtype)
